# revision 27
# baseline (speedup 1.0000x reference)
"""Trainium2 Bass kernel for nn_Attn (B=32, S=4096, H=1024, D=2*H=2048).

Reference computation:
    tmp      = einsum("bsd,hd->bsh", encoder_outputs, W) + b      # [B,S,H]
    energies = einsum("bh,bsh->bs", hidden, tmp)                  # [B,S]
    attn     = softmax(energies, axis=-1)[:, None, :]             # [B,1,S]

Key reassociation (exact in real arithmetic):
    energies[b,s] = enc[b,s,:] . v[b,:] + (hidden[b] . bias)
    with v[b,:] = hidden[b,:] @ W        # [B, D]
The bias term is constant over s, so it cancels inside softmax and is
dropped entirely.  This turns a 550-GFLOP dense matmul problem into a
memory-bound weighted-reduction stream over the 1 GiB encoder_outputs.

Sharding: data-parallel over batch across 8 cores (4 batches/core),
W replicated.  (A tensor-parallel W-shard + 32 KiB AllToAll for v was
tried and is ~20 us better on paper, but the NRT collective's prelude
barrier surfaces ~30-50 us of cross-core NEFF launch skew, which eats
the win -- so no collectives.)  Per core:
  1. W streams through a 4-deep k-tile pool at the head of the sync
     HWDGE queue (back-to-back DMAs; the 4-deep pipeline keeps the
     FIFO from stalling the enc stream behind it) and v = hidden @ W
     accumulates on TensorE as tiles land,
  2. v[b] rows move to partition-base-0 tiles (batch 0 straight out of
     PSUM partition 0 via ScalarE -- fastest path, it gates the DVE
     start; batches 1-3 via SBUF->SBUF DMA, needed much later) and are
     broadcast to 128 partitions with a rank-1 TensorE matmul
     (ones[1,128] outer v[b]) -- no DRAM roundtrip,
  3. enc tiles [128 s-partitions x 2 x 2048 d] stream on the sync
     queue (7 x 2 MiB buffers) and reduce on DVE with fused
     scalar_tensor_tensor (out = in0 * in1, accum_out = row-sum)
     against the broadcast v,
  4. softmax runs per batch as soon as that batch's stream finishes
     (overlapped with later batches' streaming), entirely in the
     [128, 32] energy layout: per-partition max/exp/sum on DVE/ScalarE,
     cross-partition max/sum via TensorE transpose-with-identity, and
     scalar broadcasts via ones-matmul,
  5. each batch's attn [128, 32] tile DMAs straight to out[b] on the
     SWDGE queue (keeps the sync queue pure enc streaming).
"""

import numpy as np

import concourse.bacc as bacc
import concourse.tile as tile
from concourse import mybir
from concourse.bass_utils import run_bass_kernel_spmd

F32 = mybir.dt.float32

B, S, H, D = 32, 4096, 1024, 2048
NCORES = 8
BL = B // NCORES          # batches per core = 4
KT = H // 128             # hidden k-tiles = 8
NJ = D // 512             # 512-wide N chunks in D = 4
SJ = 2                    # s-rows per partition per streamed DMA chunk
NQ = S // (128 * SJ)      # streamed DMA chunks per batch = 16
SCOLS = S // 128          # energy columns per partition = 32
STREAM_BUFS = 7
W_BUFS = 5


def build_bass():
    nc = bacc.Bacc()
    # hT[p, k*BL + m] = hidden_loc[m, k*128 + p]  (per-core batches)
    hT = nc.dram_tensor("hT", [128, KT * BL], F32, kind="ExternalInput")
    W = nc.dram_tensor("W", [H, D], F32, kind="ExternalInput")
    enc = nc.dram_tensor("enc", [BL, S, D], F32, kind="ExternalInput")
    ident = nc.dram_tensor("ident", [128, 128], F32, kind="ExternalInput")
    out = nc.dram_tensor("out", [BL, S], F32, kind="ExternalOutput")

    with tile.TileContext(nc) as tc:
        with (
            tc.tile_pool(name="persist", bufs=1) as persist,
            tc.tile_pool(name="wpool", bufs=W_BUFS) as wpool,
            tc.tile_pool(name="stream", bufs=STREAM_BUFS) as stream,
            tc.tile_pool(name="psum_v", bufs=1, space="PSUM") as psum_v_pool,
            tc.tile_pool(name="psum_b", bufs=2, space="PSUM") as psum_b_pool,
            tc.tile_pool(name="psum_s", bufs=2, space="PSUM") as psum_s_pool,
        ):
            # ---- small loads first on the sync queue ----
            hT_sb = persist.tile([128, KT * BL], F32, tag="hT")
            nc.sync.dma_start(out=hT_sb, in_=hT[:, :])
            # memset instead of a DMA: its completion gates the PE warm
            # matmul, and an immediate DVE memset lets the PE's ~8.5 us
            # first-instruction boot start at ~0.5 us instead of ~3.4.
            ones_sb = persist.tile([1, 128], F32, tag="ones")
            nc.vector.memset(ones_sb[:, :], 1.0)

            # ---- dummy matmul to absorb the PE sequencer's ~8 us first-
            # dispatch latency while W is still loading ----
            for _ in range(2):
                warm = psum_s_pool.tile([1, 128], F32, tag="tr")
                nc.tensor.matmul(
                    warm, ones_sb[:, 0:1], ones_sb, start=True, stop=True
                )

            # ---- v = hidden_loc @ W -> psum [BL, D], W cycled via pool ----
            psv = psum_v_pool.tile([BL, D], F32, tag="psv")
            for k in range(KT):
                wt = wpool.tile([128, D], F32, tag="w", name=f"w{k}")
                nc.sync.dma_start(out=wt, in_=W[k * 128:(k + 1) * 128, :])
                for j in range(NJ):
                    nc.tensor.matmul(
                        psv[:, j * 512:(j + 1) * 512],
                        hT_sb[:, k * BL:(k + 1) * BL],
                        wt[:, j * 512:(j + 1) * 512],
                        start=(k == 0),
                        stop=(k == KT - 1),
                    )
            # ident is first needed by the softmax transposes (~100 us in);
            # loading it after W lets w0 land a touch earlier.
            ident_sb = persist.tile([128, 128], F32, tag="ident")
            nc.sync.dma_start(out=ident_sb, in_=ident[:, :])

            # batch 0 fast path: partition 0 of PSUM is legal for ACT, so
            # copy the v row straight out of psv -- no v_sb / SWDGE hops.
            # Chunked: copy j waits only on its own chunk's k=7 stop, so it
            # overlaps the remaining psv matmuls instead of waiting for all.
            vr0 = persist.tile([1, D], F32, tag="vr0")
            for j in range(NJ):
                nc.vector.tensor_copy(
                    out=vr0[:, j * 512:(j + 1) * 512],
                    in_=psv[0:1, j * 512:(j + 1) * 512],
                )

            # batches 1-3 are needed much later: go via v_sb + SBUF->SBUF
            # DMA (engines can't touch partition offsets 1..3, DMA can).
            v_sb = persist.tile([BL, D], F32, tag="vsb")

            # Broadcast helper: v[b] row -> [128, D] via rank-1 matmul.
            # copy_fn: DVE for b0 (it gates the DVE start and DVE is idle
            # pre-stream); ACT for b1-3 so they can't stall the streaming
            # reduction.  All v-setup sits before the stream loop: ACT ops
            # placed mid-stream would block the ACT-ring enc triggers
            # behind them and make odd tiles lag their in-order
            # consumption slot on DVE (measured as periodic 3-4 us DVE
            # stalls when tried).
            def bcast_v(vr, vb, copy_fn):
                for j in range(NJ):
                    pb = psum_b_pool.tile([128, 512], F32, tag="pbc")
                    nc.tensor.matmul(
                        pb,
                        ones_sb,
                        vr[:, j * 512:(j + 1) * 512],
                        start=True,
                        stop=True,
                    )
                    copy_fn(out=vb[:, j * 512:(j + 1) * 512], in_=pb)

            v_bc = [
                persist.tile([128, D], F32, tag=f"vb{b}", name=f"vb{b}")
                for b in range(BL)
            ]
            bcast_v(vr0, v_bc[0], nc.vector.tensor_copy)

            # v setup for batches 1-3 (needed only from ~1/4 through the
            # stream; SWDGE row extraction + ACT-side broadcast copies)
            nc.scalar.copy(out=v_sb, in_=psv)
            for b2 in range(1, BL):
                # vr0 is dead once batch b2-1's broadcasts have read it;
                # the WAR dep on those matmuls orders the reuse correctly.
                nc.gpsimd.dma_start(out=vr0, in_=v_sb[b2:b2 + 1, :])
                bcast_v(vr0, v_bc[b2], nc.scalar.copy)

            # ---- stream enc, fused multiply + row-reduce on DVE ----
            # s = p*SCOLS + q*SJ + j   (p = partition, column c = q*SJ + j)
            enc_r = enc[:, :, :].rearrange(
                "b (p q j) d -> b q p j d", p=128, q=NQ, j=SJ
            )
            e_tiles = [
                persist.tile([128, SCOLS], F32, tag=f"e{b}", name=f"e{b}")
                for b in range(BL)
            ]
            for b in range(BL):
                for q in range(NQ):
                    t = stream.tile([128, SJ, D], F32, tag="enc", name="enc_t")
                    # Dual HWDGE rings (SP + ACT) for descriptor-supply
                    # parallelism; both feed the same 16 SDMA engines.
                    # ACT-ring triggers dispatch in order with ACT compute
                    # (v-setup copies pre-stream, softmax exp at batch
                    # boundaries), so ACT carries only every 4th tile of
                    # batches 1-3 -- each has ~3 tiles of delivery slack
                    # against its in-order DVE consumption slot.  A 1:1
                    # split measurably stalls DVE ~1 us per tile pair as
                    # ring phase lag compounds across batches.
                    use_act = b > 0 and q % 4 == 1
                    dma_eng = nc.scalar if use_act else nc.sync
                    dma_eng.dma_start(out=t, in_=enc_r[b, q])
                    for j in range(SJ):
                        # Fused multiply + add-reduce in one pass:
                        # out = (in0 * 1.0) * in1, accum_out = sum(out).
                        # out aliases in0 (the product is dead after the
                        # reduce).  NB: tensor_tensor_reduce wedges the device
                        # on this runtime path; scalar_tensor_tensor is the
                        # plain TENSOR_SCALAR_PTR ISA op and works.
                        nc.vector.scalar_tensor_tensor(
                            out=t[:, j, :],
                            in0=t[:, j, :],
                            scalar=1.0,
                            in1=v_bc[b],
                            op0=mybir.AluOpType.mult,
                            op1=mybir.AluOpType.mult,
                            accum_out=e_tiles[b][:, q * SJ + j:q * SJ + j + 1],
                        )

                # ---- per-batch softmax in the [128, SCOLS] layout,
                # overlapped with the next batch's streaming.
                # attn = exp(e - m_p) * exp(m_p - M) / sum, with the
                # cross-partition max/sum done on a transposed [1, 128]
                # row (negate-fused reduces; PE transpose-with-identity;
                # rank-1 matmul to return the factor to 128 partitions).
                e = e_tiles[b]
                m_n = persist.tile([128, 1], F32, tag=f"mn{b}")
                nc.vector.tensor_reduce(
                    out=m_n, in_=e, axis=mybir.AxisListType.X,
                    op=mybir.AluOpType.max, negate=True,
                )
                s_p = persist.tile([128, 1], F32, tag=f"sp{b}")
                # e <- exp(e - m_p), s_p = row sums
                nc.scalar.activation(
                    out=e, in_=e,
                    func=mybir.ActivationFunctionType.Exp,
                    bias=m_n, scale=1.0, accum_out=s_p,
                )
                # transpose -m_p and S_p to [1, 128] rows
                mTn = psum_s_pool.tile([1, 128], F32, tag="tr")
                nc.tensor.transpose(mTn, m_n, ident_sb)
                sT = psum_s_pool.tile([1, 128], F32, tag="tr")
                nc.tensor.transpose(sT, s_p, ident_sb)
                # -M = min_p(-m_p); w = exp(m_p - M) as a row
                nmx = persist.tile([1, 1], F32, tag="nmx")
                nc.vector.tensor_reduce(
                    out=nmx, in_=mTn, axis=mybir.AxisListType.X,
                    op=mybir.AluOpType.min,
                )
                wT = persist.tile([1, 128], F32, tag="wt")
                nc.scalar.activation(
                    out=wT, in_=mTn,
                    func=mybir.ActivationFunctionType.Exp,
                    bias=nmx, scale=-1.0,
                )
                # D = sum_p S_p * w_p ; f = w / D
                dscr = persist.tile([1, 128], F32, tag="dsc")
                dsum = persist.tile([1, 1], F32, tag="ds")
                nc.vector.scalar_tensor_tensor(
                    out=dscr, in0=sT, scalar=1.0, in1=wT,
                    op0=mybir.AluOpType.mult, op1=mybir.AluOpType.mult,
                    accum_out=dsum,
                )
                rden = persist.tile([1, 1], F32, tag="rd")
                nc.vector.reciprocal(out=rden, in_=dsum)
                f_row = persist.tile([1, 128], F32, tag="fr")
                nc.vector.tensor_scalar_mul(f_row, wT, rden)
                # factor back to [128, 1] via rank-1 matmul with a 1-col
                fT = psum_s_pool.tile([128, 1], F32, tag="tr")
                nc.tensor.matmul(
                    fT, f_row, ones_sb[:, 0:1], start=True, stop=True
                )
                nc.vector.tensor_scalar_mul(e, e, fT)
                # out[b, p*SCOLS + c] = e[p, c]; SWDGE queue keeps the sync
                # queue pure enc streaming.
                # b<3 on SWDGE (keeps both HWDGE rings pure enc mid-
                # stream); the last batch rides the by-then-empty sync
                # ring for its lower completion latency.
                out_eng = nc.sync if b == BL - 1 else nc.gpsimd
                out_eng.dma_start(out=out[b:b + 1, :], in_=e[:, :])

    nc.compile()
    return nc


_NC_CACHE = None


def _get_nc():
    global _NC_CACHE
    if _NC_CACHE is None:
        _NC_CACHE = build_bass()
    return _NC_CACHE


def _make_in_maps(hidden, encoder_outputs, W):
    hidden = np.asarray(hidden, dtype=np.float32)
    encoder_outputs = np.asarray(encoder_outputs, dtype=np.float32)
    W = np.ascontiguousarray(np.asarray(W, dtype=np.float32))
    ident = np.eye(128, dtype=np.float32)
    in_maps = []
    for c in range(NCORES):
        hs = hidden[c * BL:(c + 1) * BL]                       # [BL, H]
        hT = np.ascontiguousarray(
            hs.T.reshape(KT, 128, BL).transpose(1, 0, 2).reshape(128, KT * BL)
        )
        in_maps.append({
            "hT": hT,
            "W": W,
            "enc": np.ascontiguousarray(encoder_outputs[c * BL:(c + 1) * BL]),
            "ident": ident,
        })
    return in_maps


def run_device(hidden, encoder_outputs, W, trace=False, **spmd_kwargs):
    nc = _get_nc()
    in_maps = _make_in_maps(hidden, encoder_outputs, W)
    res = run_bass_kernel_spmd(
        nc, in_maps, core_ids=list(range(NCORES)), trace=trace, **spmd_kwargs
    )
    outs = np.concatenate([r["out"] for r in res.results], axis=0)  # [B, S]
    return outs[:, None, :].astype(np.float32), res


def kernel(hidden, encoder_outputs, W, b):
    # `b` (the Linear bias) shifts every energy in a row equally
    # (hidden[b].bias, independent of s), so it cancels in the softmax.
    out, _ = run_device(hidden, encoder_outputs, W)
    return out
